# revision 11
# baseline (speedup 1.0000x reference)
"""Trainium2 Bass kernel for AudioAttentionMapGenerator.

Math (reference):
    sigma = exp(log_sigma); c = 0.5 / (sigma^2 + 1e-6)
    w_n   = attn_weights * mask
    map[b,h,w] = sum_n w_n * exp(-c*((h-v_bn)^2 + (w-u_bn)^2))
    out = map / (max_hw(map) + 1e-6)

The Gaussian is separable: per sample  map = Gy^T @ (w * Gx)  — one
(H,N)@(N,W) matmul.  On-device formulation per gaussian axis:
    exp(-c (x-u)^2) * w = Exp( cneg*(x^2 - 2 u x)  +  [cneg*u^2 + ln w] )
with cneg = -c, so the whole weighted gaussian row is ONE scalar_tensor_tensor
(d = grid*(-2u) + grid2) followed by ONE activation (scale=cneg, bias folded).
The weight is folded into the x-side bias via ln(w).

Sharding: data-parallel over B=16 across 8 cores (2 samples/core).
N=128 lives on SBUF partitions; H splits 128+96 for the two matmul chunks.
Matmuls run as float32r with the moving free dim padded to 256 (full-rate).
Per-sample max: free-dim reduces (DVE) + partition all-reduce (GPSIMD).
"""

import sys

import numpy as np

if "/opt/trn_rl_repo" not in sys.path:
    sys.path.insert(0, "/opt/trn_rl_repo")

B, N, H, W = 16, 128, 224, 224
NCORES = 8
BPC = B // NCORES  # samples per core
H0 = 128  # first H chunk (matmul stationary free-dim max)
H1 = H - H0  # 96
WP = 256  # moving operand padded width (float32r full-rate needs >=256)

_CACHE = {}


def _build():
    if "nc" in _CACHE:
        return _CACHE["nc"]

    from contextlib import ExitStack

    import concourse.bass_isa as bass_isa
    import concourse.tile as tile
    from concourse import bacc, mybir

    f32 = mybir.dt.float32
    f32r = mybir.dt.float32r
    AF = mybir.ActivationFunctionType
    AX = mybir.AxisListType

    nc = bacc.Bacc(
        "TRN2",
        target_bir_lowering=False,
        debug=False,
        enable_asserts=False,
        num_devices=NCORES,
    )
    # packed per-core input: [u0, u1, v0, v1, aw0, aw1, log_sigma, 0]
    pk = nc.dram_tensor("pk", (N, 8), f32, kind="ExternalInput").ap()
    mk = nc.dram_tensor("mk", (BPC, N), mybir.dt.uint8, kind="ExternalInput").ap()
    out = nc.dram_tensor("out", (BPC, H, W), f32, kind="ExternalOutput").ap()

    gridv = np.arange(W, dtype=np.float32)
    grid_c = nc.inline_tensor(np.broadcast_to(gridv, (128, W)), "gridc").ap()
    grid2_c = nc.inline_tensor(np.broadcast_to(gridv * gridv, (128, W)), "grid2c").ap()

    with ExitStack() as ctx:
        tc = ctx.enter_context(tile.TileContext(nc))
        consts = ctx.enter_context(tc.tile_pool(name="consts", bufs=1))
        work = ctx.enter_context(tc.tile_pool(name="work", bufs=2))
        small = ctx.enter_context(tc.tile_pool(name="small", bufs=4))
        psum = ctx.enter_context(tc.tile_pool(name="psum", bufs=2, space="PSUM"))

        # ---- constants / per-core inputs ----
        grid = consts.tile([128, W], f32)
        nc.sync.dma_start(out=grid, in_=grid_c)
        grid2 = consts.tile([128, W], f32)
        nc.sync.dma_start(out=grid2, in_=grid2_c)
        pkt = consts.tile([128, 8], f32)
        nc.sync.dma_start(out=pkt, in_=pk)
        mkt = consts.tile([128, BPC], mybir.dt.uint8)
        nc.sync.dma_start(out=mkt, in_=mk.rearrange("b n -> n b"))

        # cneg = -0.5 / (exp(2*log_sigma) + 1e-6), replicated on all partitions
        sig2 = consts.tile([128, 1], f32)
        nc.scalar.activation(sig2, pkt[:, 6:7], AF.Exp, scale=2.0)
        sig2e = consts.tile([128, 1], f32)
        nc.vector.tensor_scalar_add(sig2e, sig2, 1e-6)
        rinv = consts.tile([128, 1], f32)
        nc.vector.reciprocal(rinv, sig2e)
        cneg = consts.tile([128, 1], f32)
        nc.vector.tensor_scalar_mul(cneg, rinv, -0.5)

        # weights = attn * mask;  lnw = ln(weights)
        mkf = consts.tile([128, BPC], f32)
        nc.vector.tensor_copy(mkf, mkt)
        wt = consts.tile([128, BPC], f32)
        nc.vector.tensor_mul(wt, pkt[:, 4:6], mkf)
        lnw = consts.tile([128, BPC], f32)
        nc.scalar.activation(lnw, wt, AF.Ln)

        # n2 = -2 * [u0,u1,v0,v1];  bias4 = cneg*coord^2 (+ lnw on x slots)
        n2 = consts.tile([128, 4], f32)
        nc.vector.tensor_scalar_mul(n2, pkt[:, 0:4], -2.0)
        pc2 = consts.tile([128, 4], f32)
        nc.vector.tensor_mul(pc2, pkt[:, 0:4], pkt[:, 0:4])
        bias4 = consts.tile([128, 4], f32)
        nc.vector.tensor_scalar(bias4, pc2, cneg[:, 0:1], None, mybir.AluOpType.mult)
        nc.vector.tensor_add(bias4[:, 0:2], bias4[:, 0:2], lnw)

        # ---- per-sample pipeline ----
        for b in range(BPC):
            # x side (carries the weight): one STT + one Exp into padded tile
            ddx = work.tile([128, W], f32, tag="ddx")
            nc.vector.scalar_tensor_tensor(
                ddx, grid, n2[:, b : b + 1], grid2,
                mybir.AluOpType.mult, mybir.AluOpType.add,
            )
            gx = work.tile([128, WP], f32r, tag="gx")
            nc.vector.memset(gx[:, W:WP].bitcast(mybir.dt.uint32), 0)
            nc.scalar.activation(
                gx[:, 0:W], ddx, AF.Exp, scale=cneg[:, 0:1], bias=bias4[:, b : b + 1]
            )
            # y side
            ddy = work.tile([128, W], f32, tag="ddy")
            nc.vector.scalar_tensor_tensor(
                ddy, grid, n2[:, 2 + b : 3 + b], grid2,
                mybir.AluOpType.mult, mybir.AluOpType.add,
            )
            gy = work.tile([128, W], f32r, tag="gy")
            nc.scalar.activation(
                gy, ddy, AF.Exp, scale=cneg[:, 0:1], bias=bias4[:, 2 + b : 3 + b]
            )

            pmap0 = psum.tile([H0, WP], f32, tag="pmap0")
            pmap1 = psum.tile([H1, WP], f32, tag="pmap1")
            nc.tensor.matmul(pmap0, gy[:, 0:H0], gx)
            nc.tensor.matmul(pmap1, gy[:, H0:H], gx)

            # per-sample max over the whole map
            mcol = small.tile([128, 2], f32, tag="mcol")
            nc.vector.memset(mcol, 0.0)
            nc.vector.reduce_max(mcol[:, 0:1], pmap0[:, 0:W], axis=AX.X)
            nc.vector.reduce_max(mcol[0:H1, 1:2], pmap1[:, 0:W], axis=AX.X)
            mrow = small.tile([128, 1], f32, tag="mrow")
            nc.vector.reduce_max(mrow, mcol, axis=AX.X)
            mall = small.tile([128, 1], f32, tag="mall")
            nc.gpsimd.partition_all_reduce(
                mall, mrow, channels=128, reduce_op=bass_isa.ReduceOp.max
            )
            mxe = small.tile([128, 1], f32, tag="mxe")
            nc.vector.tensor_scalar_add(mxe, mall, 1e-6)
            rs = small.tile([128, 1], f32, tag="rs")
            nc.vector.reciprocal(rs, mxe)

            o0 = work.tile([H0, W], f32, tag="o0")
            nc.scalar.mul(o0, pmap0[:, 0:W], rs[:H0, 0:1])
            o1 = work.tile([H1, W], f32, tag="o1")
            nc.vector.tensor_scalar_mul(o1, pmap1[:, 0:W], rs[:H1, 0:1])
            nc.sync.dma_start(out=out[b, 0:H0, :], in_=o0)
            nc.sync.dma_start(out=out[b, H0:H, :], in_=o1)

    nc.compile()
    _CACHE["nc"] = nc
    return nc


def kernel(pixel_coords, attn_weights, in_frame_mask, log_sigma, **kwargs):
    pixel_coords = np.asarray(pixel_coords, dtype=np.float32)
    attn_weights = np.asarray(attn_weights, dtype=np.float32)
    mask_u8 = np.ascontiguousarray(np.asarray(in_frame_mask).astype(np.uint8))
    ls = float(np.asarray(log_sigma, dtype=np.float32))

    nc = _build()
    from concourse.bass_utils import run_bass_kernel_spmd

    in_maps = []
    for i in range(NCORES):
        sl = slice(i * BPC, (i + 1) * BPC)
        pc = pixel_coords[sl]  # (BPC, N, 2)
        aw = attn_weights[sl]  # (BPC, N)
        pkt = np.empty((N, 8), dtype=np.float32)
        pkt[:, 0] = pc[0, :, 0]
        pkt[:, 1] = pc[1, :, 0]
        pkt[:, 2] = pc[0, :, 1]
        pkt[:, 3] = pc[1, :, 1]
        pkt[:, 4] = aw[0]
        pkt[:, 5] = aw[1]
        pkt[:, 6] = ls
        pkt[:, 7] = 0.0
        in_maps.append({"pk": pkt, "mk": mask_u8[sl]})
    res = run_bass_kernel_spmd(nc, in_maps, core_ids=list(range(NCORES)))
    return np.concatenate([r["out"] for r in res.results], axis=0)


# revision 12
# speedup vs baseline: 1.0408x; 1.0408x over previous
"""Trainium2 Bass kernel for AudioAttentionMapGenerator.

Math (reference):
    sigma = exp(log_sigma); c = 0.5 / (sigma^2 + 1e-6)
    w_n   = attn_weights * mask
    map[b,h,w] = sum_n w_n * exp(-c*((h-v_bn)^2 + (w-u_bn)^2))
    out = map / (max_hw(map) + 1e-6)

The Gaussian is separable: per sample  map = Gy^T @ (w * Gx)  — one
(H,N)@(N,W) matmul.  Per gaussian axis the exponent is expanded as
    -c (x-u)^2 = cneg*(x^2 - 2 u x) + cneg*u^2
so each gaussian row block is ONE scalar_tensor_tensor
(d = grid*(-2u) + grid2; grid/grid2 are inline NEFF constants) followed by
ONE activation Exp(scale=cneg, bias=cneg*u^2).

Sharding: data-parallel over B=16 across 8 cores (2 samples/core).
N=128 lives on SBUF partitions; H splits 112+112 so both map chunks live in
one PSUM tile and one output DMA per sample covers both.
Matmuls run as float32r with the moving free dim padded to 256 (full rate).
Per-sample max: free-dim reduces (DVE) + partition all-reduce (GPSIMD).
"""

import sys

import numpy as np

if "/opt/trn_rl_repo" not in sys.path:
    sys.path.insert(0, "/opt/trn_rl_repo")

B, N, H, W = 16, 128, 224, 224
NCORES = 8
BPC = B // NCORES  # samples per core
HC = H // 2  # 112 — H chunk (stationary free-dim <= 128)
WP = 256  # moving operand padded width (float32r full-rate needs >=256)

_CACHE = {}


def _build():
    if "nc" in _CACHE:
        return _CACHE["nc"]

    from contextlib import ExitStack

    import concourse.bass_isa as bass_isa
    import concourse.tile as tile
    from concourse import bacc, mybir

    f32 = mybir.dt.float32
    f32r = mybir.dt.float32r
    AF = mybir.ActivationFunctionType
    AX = mybir.AxisListType
    OP = mybir.AluOpType

    nc = bacc.Bacc(
        "TRN2",
        target_bir_lowering=False,
        debug=False,
        enable_asserts=False,
        num_devices=NCORES,
    )
    # packed per-core input: [u0,u1,v0,v1, aw0,aw1, m0,m1, log_sigma, pad...]
    pk = nc.dram_tensor("pk", (N, 12), f32, kind="ExternalInput").ap()
    out = nc.dram_tensor("out", (BPC, H, W), f32, kind="ExternalOutput").ap()

    gridv = np.arange(W, dtype=np.float32)
    gg = np.concatenate([gridv, gridv * gridv]).astype(np.float32)
    gg_c = nc.inline_tensor(np.broadcast_to(gg, (128, 2 * W)).copy(), "ggc").ap()

    with ExitStack() as ctx:
        tc = ctx.enter_context(tile.TileContext(nc))
        consts = ctx.enter_context(tc.tile_pool(name="consts", bufs=1))
        work = ctx.enter_context(tc.tile_pool(name="work", bufs=2))
        small = ctx.enter_context(tc.tile_pool(name="small", bufs=4))
        psum = ctx.enter_context(tc.tile_pool(name="psum", bufs=2, space="PSUM"))

        # ---- constants / per-core inputs ----
        ggt = consts.tile([128, 2, W], f32)  # [grid | grid^2]
        nc.sync.dma_start(out=ggt, in_=gg_c.rearrange("p (a w) -> p a w", a=2))
        grid = ggt[:, 0, :]
        grid2 = ggt[:, 1, :]
        pkt = consts.tile([128, 12], f32)
        nc.scalar.dma_start(out=pkt, in_=pk)

        # cneg = -0.5 / (exp(2*log_sigma) + 1e-6), replicated on all partitions
        sig2 = consts.tile([128, 1], f32)
        nc.scalar.activation(sig2, pkt[:, 8:9], AF.Exp, scale=2.0)
        sig2e = consts.tile([128, 1], f32)
        nc.vector.tensor_scalar_add(sig2e, sig2, 1e-6)
        rinv = consts.tile([128, 1], f32)
        nc.vector.reciprocal(rinv, sig2e)
        cneg = consts.tile([128, 1], f32)
        nc.vector.tensor_scalar_mul(cneg, rinv, -0.5)

        # weights = attn * mask
        wt = consts.tile([128, BPC], f32)
        nc.vector.tensor_mul(wt, pkt[:, 4:6], pkt[:, 6:8])

        # n2 = -2*[u0,u1,v0,v1];  bias4 = cneg*coord^2
        n2 = consts.tile([128, 4], f32)
        nc.vector.tensor_scalar_mul(n2, pkt[:, 0:4], -2.0)
        pc2 = consts.tile([128, 4], f32)
        nc.vector.tensor_mul(pc2, pkt[:, 0:4], pkt[:, 0:4])
        bias4 = consts.tile([128, 4], f32)
        nc.vector.tensor_scalar(bias4, pc2, cneg[:, 0:1], None, OP.mult)

        # ---- per-sample pipeline ----
        for b in range(BPC):
            # y side: stationary operand of the matmul
            ddy = work.tile([128, W], f32, tag="ddy")
            nc.vector.scalar_tensor_tensor(
                ddy, grid, n2[:, 2 + b : 3 + b], grid2, OP.mult, OP.add
            )
            gy = work.tile([128, W], f32r, tag="gy")
            nc.scalar.activation(
                gy, ddy, AF.Exp, scale=cneg[:, 0:1], bias=bias4[:, 2 + b : 3 + b]
            )
            # x side; weight applied on the way to f32r
            ddx = work.tile([128, W], f32, tag="ddx")
            nc.vector.scalar_tensor_tensor(
                ddx, grid, n2[:, b : b + 1], grid2, OP.mult, OP.add
            )
            gx = work.tile([128, W], f32, tag="gx")
            nc.scalar.activation(
                gx, ddx, AF.Exp, scale=cneg[:, 0:1], bias=bias4[:, b : b + 1]
            )
            wgx = work.tile([128, WP], f32r, tag="wgx")
            nc.vector.memset(wgx[:, W:WP].bitcast(mybir.dt.uint32), 0)
            nc.vector.tensor_scalar_mul(wgx[:, 0:W], gx, wt[:, b : b + 1])

            pmap = psum.tile([HC, 2, WP], f32, tag="pmap")
            nc.tensor.matmul(pmap[:, 0, :], gy[:, 0:HC], wgx)
            nc.tensor.matmul(pmap[:, 1, :], gy[:, HC:H], wgx)

            # per-sample max over the whole map
            mcol = small.tile([HC, 2], f32, tag="mcol")
            nc.vector.reduce_max(mcol[:, 0:1], pmap[:, 0, 0:W], axis=AX.X)
            nc.vector.reduce_max(mcol[:, 1:2], pmap[:, 1, 0:W], axis=AX.X)
            mrow = small.tile([HC, 1], f32, tag="mrow")
            nc.vector.reduce_max(mrow, mcol, axis=AX.X)
            mall = small.tile([HC, 1], f32, tag="mall")
            nc.gpsimd.partition_all_reduce(
                mall, mrow, channels=HC, reduce_op=bass_isa.ReduceOp.max
            )
            mxe = small.tile([HC, 1], f32, tag="mxe")
            nc.vector.tensor_scalar_add(mxe, mall, 1e-6)
            rs = small.tile([HC, 1], f32, tag="rs")
            nc.vector.reciprocal(rs, mxe)

            o_all = work.tile([HC, 2, W], f32, tag="o_all")
            nc.scalar.mul(o_all[:, 0, :], pmap[:, 0, 0:W], rs[:, 0:1])
            nc.vector.tensor_scalar_mul(o_all[:, 1, :], pmap[:, 1, 0:W], rs[:, 0:1])
            nc.sync.dma_start(
                out=out[b].rearrange("(c p) w -> p c w", c=2), in_=o_all
            )

    nc.compile()
    _CACHE["nc"] = nc
    return nc


def kernel(pixel_coords, attn_weights, in_frame_mask, log_sigma, **kwargs):
    pixel_coords = np.asarray(pixel_coords, dtype=np.float32)
    attn_weights = np.asarray(attn_weights, dtype=np.float32)
    mask_f = np.asarray(in_frame_mask).astype(np.float32)
    ls = float(np.asarray(log_sigma, dtype=np.float32))

    nc = _build()
    from concourse.bass_utils import run_bass_kernel_spmd

    in_maps = []
    for i in range(NCORES):
        sl = slice(i * BPC, (i + 1) * BPC)
        pc = pixel_coords[sl]  # (BPC, N, 2)
        aw = attn_weights[sl]  # (BPC, N)
        mf = mask_f[sl]
        pkt = np.zeros((N, 12), dtype=np.float32)
        pkt[:, 0] = pc[0, :, 0]
        pkt[:, 1] = pc[1, :, 0]
        pkt[:, 2] = pc[0, :, 1]
        pkt[:, 3] = pc[1, :, 1]
        pkt[:, 4] = aw[0]
        pkt[:, 5] = aw[1]
        pkt[:, 6] = mf[0]
        pkt[:, 7] = mf[1]
        pkt[:, 8] = ls
        in_maps.append({"pk": pkt})
    res = run_bass_kernel_spmd(nc, in_maps, core_ids=list(range(NCORES)))
    return np.concatenate([r["out"] for r in res.results], axis=0)


# revision 41
# speedup vs baseline: 1.2127x; 1.1652x over previous
"""Trainium2 Bass kernel for AudioAttentionMapGenerator.

Math (reference):
    sigma = exp(log_sigma); c = 0.5 / (sigma^2 + 1e-6)
    w_n   = attn_weights * mask
    map[b,h,w] = sum_n w_n * exp(-c*((h-v_bn)^2 + (w-u_bn)^2))
    out = map / (max_hw(map) + 1e-6)

The Gaussian is separable: per sample  map = Gy^T @ (w * Gx)  — one
(H,N)@(N,W) matmul.  Per gaussian axis the exponent is expanded as
    -c (x-u)^2 = cneg*(x^2 - 2 u x) + cneg*u^2
so each gaussian row block is ONE scalar_tensor_tensor
(d = grid*(-2u) + grid2; grid/grid2 are inline NEFF constants) followed by
ONE activation Exp(scale=cneg, bias=cneg*u^2).

Sharding: data-parallel over B=16 across 8 cores (2 samples/core).
N=128 lives on SBUF partitions; H splits 112+112 so both map chunks live in
one PSUM tile and one output DMA per sample covers both.
Matmuls run as float32r with the moving free dim padded to 256 (full rate).
Per-sample max: free-dim reduces (DVE) + partition all-reduce (GPSIMD).
"""

import sys

import numpy as np

if "/opt/trn_rl_repo" not in sys.path:
    sys.path.insert(0, "/opt/trn_rl_repo")

B, N, H, W = 16, 128, 224, 224
NCORES = 8
BPC = B // NCORES  # samples per core
HC = H // 2  # 112 — H chunk (stationary free-dim <= 128)
WP = 256  # moving operand padded width (float32r full-rate needs >=256)

_CACHE = {}


def _build():
    if "nc" in _CACHE:
        return _CACHE["nc"]

    from contextlib import ExitStack

    import concourse.bass_isa as bass_isa
    import concourse.tile as tile
    from concourse import bacc, mybir

    f32 = mybir.dt.float32
    f32r = mybir.dt.float32r
    AF = mybir.ActivationFunctionType
    AX = mybir.AxisListType
    OP = mybir.AluOpType

    nc = bacc.Bacc(
        "TRN2",
        target_bir_lowering=False,
        debug=False,
        enable_asserts=False,
        num_devices=NCORES,
    )
    # packed per-core input: [u0,u1,v0,v1, aw0,aw1, m0,m1, log_sigma, pad...]
    pk = nc.dram_tensor("pk", (N, 12), f32, kind="ExternalInput").ap()
    out = nc.dram_tensor("out", (BPC, H, W), f32, kind="ExternalOutput").ap()

    from concourse.tile import add_dep_helper

    with ExitStack() as ctx:
        tc = ctx.enter_context(tile.TileContext(nc))
        consts = ctx.enter_context(tc.tile_pool(name="consts", bufs=1))
        work = ctx.enter_context(tc.tile_pool(name="work", bufs=2))
        small = ctx.enter_context(tc.tile_pool(name="small", bufs=4))
        psum = ctx.enter_context(tc.tile_pool(name="psum", bufs=2, space="PSUM"))

        # ---- constants / per-core inputs ----
        pkt = consts.tile([128, 12], f32)
        nc.sync.dma_start(out=pkt, in_=pk)
        # grid = [0..W) per partition, generated on-chip (prefix scan of ones)
        # so no DMA sits on the critical path; grid2 = grid^2
        ones = consts.tile([128, W], f32)
        nc.vector.memset(ones, 1.0)
        grid = consts.tile([128, W], f32)
        nc.vector.tensor_tensor_scan(grid, ones, ones, -1.0, OP.add, OP.mult)
        grid2 = consts.tile([128, W], f32)
        nc.vector.tensor_mul(grid2, grid, grid)

        # cneg = -0.5 / (exp(2*log_sigma) + 1e-6), replicated on all partitions:
        # reciprocal((sig2 + 1e-6) * -2)
        sig2 = consts.tile([128, 1], f32)
        nc.scalar.activation(sig2, pkt[:, 8:9], AF.Exp, scale=2.0)
        sig2e = consts.tile([128, 1], f32)
        nc.vector.tensor_scalar(sig2e, sig2, 1e-6, -2.0, OP.add, OP.mult)
        cneg = consts.tile([128, 1], f32)
        i_cneg = nc.vector.reciprocal(cneg, sig2e)

        # weights = attn * mask
        wt = consts.tile([128, BPC], f32)
        nc.vector.tensor_mul(wt, pkt[:, 4:6], pkt[:, 6:8])

        # n2 = -2*[u0,u1,v0,v1];  ncrd = -coords;  bias4 = cneg*coord^2
        n2 = consts.tile([128, 4], f32)
        nc.vector.tensor_scalar_mul(n2, pkt[:, 0:4], -2.0)
        ncrd = consts.tile([128, 4], f32)
        nc.vector.tensor_scalar_mul(ncrd, pkt[:, 0:4], -1.0)
        pc2 = consts.tile([128, 4], f32)
        nc.vector.tensor_mul(pc2, pkt[:, 0:4], pkt[:, 0:4])
        bias4 = consts.tile([128, 4], f32)
        i_bias4 = nc.vector.tensor_scalar(bias4, pc2, cneg[:, 0:1], None, OP.mult)

        # ---- per-sample pipeline ----
        # per-engine staging: ACT writes chunk-0 rows, DVE writes chunk-1 rows;
        # one DMA per staging tile (two total), so no shared-writer stalls
        o_act = consts.tile([HC, BPC, W], f32)
        o_dve = consts.tile([HC, BPC, W], f32)
        prev = {}
        for b in range(BPC):
            # x side first; weight applied on the way to f32r (DVE) while the
            # y-side Exp runs on ACT
            ddx = work.tile([128, W], f32, tag="ddx")
            i_sx = nc.vector.scalar_tensor_tensor(
                ddx, grid, n2[:, b : b + 1], grid2, OP.mult, OP.add
            )
            gx = work.tile([128, W], f32, tag="gx")
            nc.scalar.activation(
                gx, ddx, AF.Exp, scale=cneg[:, 0:1], bias=bias4[:, b : b + 1]
            )
            wgx = work.tile([128, WP], f32r, tag="wgx")
            nc.vector.memset(wgx[:, W:WP].bitcast(mybir.dt.uint32), 0)
            i_wgx = nc.vector.tensor_scalar_mul(wgx[:, 0:W], gx, wt[:, b : b + 1])
            # y side entirely on ACT: (grid - v)^2 via Square's free affine,
            # then Exp — exact, no cancellation, zero DVE ops
            ddy = work.tile([128, W], f32, tag="ddy")
            nc.scalar.activation(
                ddy, grid, AF.Square, bias=ncrd[:, 2 + b : 3 + b]
            )
            gy = work.tile([128, W], f32r, tag="gy")
            nc.scalar.activation(gy, ddy, AF.Exp, scale=cneg[:, 0:1])
            # keep the tiny cneg/bias4 chain ahead of the fat STTs in the
            # in-order DVE queue
            add_dep_helper(i_sx.ins, i_cneg.ins, sync=False, reason="cneg first")
            add_dep_helper(i_sx.ins, i_bias4.ins, sync=False, reason="bias4 first")

            pmap = psum.tile([HC, 2, WP], f32, tag="pmap")
            nc.tensor.matmul(pmap[:, 0, :], gy[:, 0:HC], wgx)
            nc.tensor.matmul(pmap[:, 1, :], gy[:, HC:H], wgx)

            # per-sample max over the whole map: free-dim reduce on DVE, then
            # partition all-reduce + combine + eps on GPSIMD (one engine hop)
            mcol = small.tile([HC, 2], f32, tag="mcol")
            i_red = nc.vector.reduce_max(mcol, pmap[:, :, 0:W], axis=AX.X)
            mrow = small.tile([HC, 1], f32, tag="mrow")
            nc.vector.reduce_max(mrow, mcol, axis=AX.X)
            mall = small.tile([HC, 1], f32, tag="mall")
            nc.gpsimd.partition_all_reduce(
                mall, mrow, channels=HC, reduce_op=bass_isa.ReduceOp.max
            )
            mxe = small.tile([HC, 1], f32, tag="mxe")
            nc.vector.tensor_scalar_add(mxe, mall, 1e-6)
            rs = small.tile([HC, 1], f32, tag="rs")
            i_recip = nc.vector.reciprocal(rs, mxe)

            nc.scalar.mul(o_act[:, b, :], pmap[:, 0, 0:W], rs[:, 0:1])
            nc.vector.tensor_scalar_mul(o_dve[:, b, :], pmap[:, 1, 0:W], rs[:, 0:1])
            prev = {"wgx": i_wgx, "recip": i_recip}
        ost = out.rearrange("b (c p) w -> p b c w", c=2)
        nc.sync.dma_start(out=ost[:, :, 0, :], in_=o_act)
        nc.sync.dma_start(out=ost[:, :, 1, :], in_=o_dve)

    nc.compile()
    _CACHE["nc"] = nc
    return nc


def kernel(pixel_coords, attn_weights, in_frame_mask, log_sigma, **kwargs):
    pixel_coords = np.asarray(pixel_coords, dtype=np.float32)
    attn_weights = np.asarray(attn_weights, dtype=np.float32)
    mask_f = np.asarray(in_frame_mask).astype(np.float32)
    ls = float(np.asarray(log_sigma, dtype=np.float32))

    nc = _build()
    from concourse.bass_utils import run_bass_kernel_spmd

    in_maps = []
    for i in range(NCORES):
        sl = slice(i * BPC, (i + 1) * BPC)
        pc = pixel_coords[sl]  # (BPC, N, 2)
        aw = attn_weights[sl]  # (BPC, N)
        mf = mask_f[sl]
        pkt = np.zeros((N, 12), dtype=np.float32)
        pkt[:, 0] = pc[0, :, 0]
        pkt[:, 1] = pc[1, :, 0]
        pkt[:, 2] = pc[0, :, 1]
        pkt[:, 3] = pc[1, :, 1]
        pkt[:, 4] = aw[0]
        pkt[:, 5] = aw[1]
        pkt[:, 6] = mf[0]
        pkt[:, 7] = mf[1]
        pkt[:, 8] = ls
        in_maps.append({"pk": pkt})
    res = run_bass_kernel_spmd(nc, in_maps, core_ids=list(range(NCORES)))
    return np.concatenate([r["out"] for r in res.results], axis=0)


# revision 45
# speedup vs baseline: 1.2315x; 1.0155x over previous
"""Trainium2 Bass kernel for AudioAttentionMapGenerator.

Math (reference):
    sigma = exp(log_sigma); c = 0.5 / (sigma^2 + 1e-6)
    w_n   = attn_weights * mask
    map[b,h,w] = sum_n w_n * exp(-c*((h-v_bn)^2 + (w-u_bn)^2))
    out = map / (max_hw(map) + 1e-6)

The Gaussian is separable: per sample  map = Gy^T @ (w * Gx)  — one
(H,N)@(N,W) matmul.  Per gaussian axis the exponent is expanded as
    -c (x-u)^2 = cneg*(x^2 - 2 u x) + cneg*u^2
so each gaussian row block is ONE scalar_tensor_tensor
(d = grid*(-2u) + grid2; grid/grid2 are inline NEFF constants) followed by
ONE activation Exp(scale=cneg, bias=cneg*u^2).

Sharding: data-parallel over B=16 across 8 cores (2 samples/core).
N=128 lives on SBUF partitions; H splits 112+112 so both map chunks live in
one PSUM tile and one output DMA per sample covers both.
Matmuls run as float32r with the moving free dim padded to 256 (full rate).
Per-sample max: free-dim reduces (DVE) + partition all-reduce (GPSIMD).
"""

import sys

import numpy as np

if "/opt/trn_rl_repo" not in sys.path:
    sys.path.insert(0, "/opt/trn_rl_repo")

B, N, H, W = 16, 128, 224, 224
NCORES = 8
BPC = B // NCORES  # samples per core
HC = H // 2  # 112 — H chunk (stationary free-dim <= 128)
WP = 256  # moving operand padded width (float32r full-rate needs >=256)

_CACHE = {}


def _build():
    if "nc" in _CACHE:
        return _CACHE["nc"]

    from contextlib import ExitStack

    import concourse.bass_isa as bass_isa
    import concourse.tile as tile
    from concourse import bacc, mybir

    f32 = mybir.dt.float32
    f32r = mybir.dt.float32r
    AF = mybir.ActivationFunctionType
    AX = mybir.AxisListType
    OP = mybir.AluOpType

    nc = bacc.Bacc(
        "TRN2",
        target_bir_lowering=False,
        debug=False,
        enable_asserts=False,
        num_devices=NCORES,
    )
    # packed per-core input: [u0,u1,v0,v1, aw0,aw1, m0,m1, log_sigma, pad...]
    pk = nc.dram_tensor("pk", (N, 12), f32, kind="ExternalInput").ap()
    out = nc.dram_tensor("out", (BPC, H, W), f32, kind="ExternalOutput").ap()

    from concourse.tile import add_dep_helper

    with ExitStack() as ctx:
        tc = ctx.enter_context(tile.TileContext(nc))
        consts = ctx.enter_context(tc.tile_pool(name="consts", bufs=1))
        work = ctx.enter_context(tc.tile_pool(name="work", bufs=2))
        small = ctx.enter_context(tc.tile_pool(name="small", bufs=4))
        psum = ctx.enter_context(tc.tile_pool(name="psum", bufs=2, space="PSUM"))

        # ---- constants / per-core inputs ----
        pkt = consts.tile([128, 12], f32)
        nc.sync.dma_start(out=pkt, in_=pk)
        # grid = [0..W) per partition, generated on-chip (prefix scan of ones)
        # so no DMA sits on the critical path; grid2 = grid^2
        ones = consts.tile([128, W], f32)
        nc.vector.memset(ones, 1.0)
        grid = consts.tile([128, W], f32)
        nc.vector.tensor_tensor_scan(grid, ones, ones, -1.0, OP.add, OP.mult)
        grid2 = consts.tile([128, W], f32)
        nc.vector.tensor_mul(grid2, grid, grid)

        # cneg = -0.5 / (exp(2*log_sigma) + 1e-6), replicated on all partitions:
        # reciprocal((sig2 + 1e-6) * -2)
        sig2 = consts.tile([128, 1], f32)
        nc.scalar.activation(sig2, pkt[:, 8:9], AF.Exp, scale=2.0)
        sig2e = consts.tile([128, 1], f32)
        nc.vector.tensor_scalar(sig2e, sig2, 1e-6, -2.0, OP.add, OP.mult)
        cneg = consts.tile([128, 1], f32)
        i_cneg = nc.vector.reciprocal(cneg, sig2e)

        # weights = attn * mask
        wt = consts.tile([128, BPC], f32)
        nc.vector.tensor_mul(wt, pkt[:, 4:6], pkt[:, 6:8])

        # n2 = -2*[u0,u1,v0,v1];  ncrd = -coords;  bias4 = cneg*coord^2
        n2 = consts.tile([128, 4], f32)
        nc.vector.tensor_scalar_mul(n2, pkt[:, 0:4], -2.0)
        ncrd = consts.tile([128, 4], f32)
        nc.vector.tensor_scalar_mul(ncrd, pkt[:, 0:4], -1.0)
        pc2 = consts.tile([128, 4], f32)
        nc.vector.tensor_mul(pc2, pkt[:, 0:4], pkt[:, 0:4])
        bias4 = consts.tile([128, 4], f32)
        i_bias4 = nc.vector.tensor_scalar(bias4, pc2, cneg[:, 0:1], None, OP.mult)

        # ---- per-sample pipeline ----
        # per-engine staging: ACT writes chunk-0 rows, DVE writes chunk-1 rows;
        # one DMA per staging tile (two total), so no shared-writer stalls
        o_act = consts.tile([HC, BPC, W], f32)
        o_dve = consts.tile([HC, BPC, W], f32)
        prev = {}
        for b in range(BPC):
            # x side first; weight applied on the way to f32r (DVE) while the
            # y-side Exp runs on ACT
            ddx = work.tile([128, W], f32, tag="ddx")
            i_sx = nc.vector.scalar_tensor_tensor(
                ddx, grid, n2[:, b : b + 1], grid2, OP.mult, OP.add
            )
            gx = work.tile([128, W], f32, tag="gx")
            nc.scalar.activation(
                gx, ddx, AF.Exp, scale=cneg[:, 0:1], bias=bias4[:, b : b + 1]
            )
            wgx = work.tile([128, WP], f32r, tag="wgx")
            nc.vector.memset(wgx[:, W:WP].bitcast(mybir.dt.uint32), 0)
            i_wgx = nc.vector.tensor_scalar_mul(wgx[:, 0:W], gx, wt[:, b : b + 1])
            # y side entirely on ACT: (grid - v)^2 via Square's free affine,
            # then Exp — exact, no cancellation, zero DVE ops
            ddy = work.tile([128, W], f32, tag="ddy")
            nc.scalar.activation(
                ddy, grid, AF.Square, bias=ncrd[:, 2 + b : 3 + b]
            )
            gy = work.tile([128, W], f32r, tag="gy")
            nc.scalar.activation(gy, ddy, AF.Exp, scale=cneg[:, 0:1])
            # keep the tiny cneg/bias4 chain ahead of the fat STTs in the
            # in-order DVE queue
            add_dep_helper(i_sx.ins, i_cneg.ins, sync=False, reason="cneg first")
            add_dep_helper(i_sx.ins, i_bias4.ins, sync=False, reason="bias4 first")

            pmap = psum.tile([HC, 2, WP], f32, tag="pmap")
            nc.tensor.matmul(pmap[:, 0, :], gy[:, 0:HC], wgx)
            nc.tensor.matmul(pmap[:, 1, :], gy[:, HC:H], wgx)

            # per-sample max over the whole map: free-dim reduce on DVE, then
            # partition all-reduce + combine + eps on GPSIMD (one engine hop)
            mrow = small.tile([HC, 1], f32, tag="mrow")
            nc.vector.reduce_max(mrow, pmap[:, :, 0:W], axis=AX.XY)
            mall = small.tile([HC, 1], f32, tag="mall")
            nc.gpsimd.partition_all_reduce(
                mall, mrow, channels=HC, reduce_op=bass_isa.ReduceOp.max
            )
            mxe = small.tile([HC, 1], f32, tag="mxe")
            nc.vector.tensor_scalar_add(mxe, mall, 1e-6)
            rs = small.tile([HC, 1], f32, tag="rs")
            i_recip = nc.vector.reciprocal(rs, mxe)

            nc.scalar.mul(o_act[:, b, :], pmap[:, 0, 0:W], rs[:, 0:1])
            nc.vector.tensor_scalar_mul(o_dve[:, b, :], pmap[:, 1, 0:W], rs[:, 0:1])
            prev = {"wgx": i_wgx, "recip": i_recip}
        ost = out.rearrange("b (c p) w -> p b c w", c=2)
        nc.sync.dma_start(out=ost[:, :, 0, :], in_=o_act)
        nc.sync.dma_start(out=ost[:, :, 1, :], in_=o_dve)

    nc.compile()
    _CACHE["nc"] = nc
    return nc


def kernel(pixel_coords, attn_weights, in_frame_mask, log_sigma, **kwargs):
    pixel_coords = np.asarray(pixel_coords, dtype=np.float32)
    attn_weights = np.asarray(attn_weights, dtype=np.float32)
    mask_f = np.asarray(in_frame_mask).astype(np.float32)
    ls = float(np.asarray(log_sigma, dtype=np.float32))

    nc = _build()
    from concourse.bass_utils import run_bass_kernel_spmd

    in_maps = []
    for i in range(NCORES):
        sl = slice(i * BPC, (i + 1) * BPC)
        pc = pixel_coords[sl]  # (BPC, N, 2)
        aw = attn_weights[sl]  # (BPC, N)
        mf = mask_f[sl]
        pkt = np.zeros((N, 12), dtype=np.float32)
        pkt[:, 0] = pc[0, :, 0]
        pkt[:, 1] = pc[1, :, 0]
        pkt[:, 2] = pc[0, :, 1]
        pkt[:, 3] = pc[1, :, 1]
        pkt[:, 4] = aw[0]
        pkt[:, 5] = aw[1]
        pkt[:, 6] = mf[0]
        pkt[:, 7] = mf[1]
        pkt[:, 8] = ls
        in_maps.append({"pk": pkt})
    res = run_bass_kernel_spmd(nc, in_maps, core_ids=list(range(NCORES)))
    return np.concatenate([r["out"] for r in res.results], axis=0)


# revision 56
# speedup vs baseline: 1.2698x; 1.0311x over previous
"""Trainium2 Bass kernel for AudioAttentionMapGenerator.

Math (reference):
    sigma = exp(log_sigma); c = 0.5 / (sigma^2 + 1e-6)
    w_n   = attn_weights * mask
    map[b,h,w] = sum_n w_n * exp(-c*((h-v_bn)^2 + (w-u_bn)^2))
    out = map / (max_hw(map) + 1e-6)

The Gaussian is separable: per sample  map = Gy^T @ (w * Gx)  — one
(H,N)@(N,W) matmul.  Per gaussian axis the exponent is expanded as
    -c (x-u)^2 = cneg*(x^2 - 2 u x) + cneg*u^2
so each gaussian row block is ONE scalar_tensor_tensor
(d = grid*(-2u) + grid2; grid/grid2 are inline NEFF constants) followed by
ONE activation Exp(scale=cneg, bias=cneg*u^2).

Sharding: data-parallel over B=16 across 8 cores (2 samples/core).
N=128 lives on SBUF partitions; H splits 112+112 so both map chunks live in
one PSUM tile and one output DMA per sample covers both.
Matmuls run as float32r with the moving free dim padded to 256 (full rate).
Per-sample max: free-dim reduces (DVE) + partition all-reduce (GPSIMD).
"""

import sys

import numpy as np

if "/opt/trn_rl_repo" not in sys.path:
    sys.path.insert(0, "/opt/trn_rl_repo")

B, N, H, W = 16, 128, 224, 224
NCORES = 8
BPC = B // NCORES  # samples per core
HC = H // 2  # 112 — H chunk (stationary free-dim <= 128)
WP = 256  # moving operand padded width (float32r full-rate needs >=256)

_CACHE = {}


def _build():
    if "nc" in _CACHE:
        return _CACHE["nc"]

    from contextlib import ExitStack

    import concourse.bass_isa as bass_isa
    import concourse.tile as tile
    from concourse import bacc, mybir

    f32 = mybir.dt.float32
    f32r = mybir.dt.float32r
    AF = mybir.ActivationFunctionType
    AX = mybir.AxisListType
    OP = mybir.AluOpType

    nc = bacc.Bacc(
        "TRN2",
        target_bir_lowering=False,
        debug=False,
        enable_asserts=False,
        num_devices=NCORES,
    )
    # packed per-core input: [u0,u1,v0,v1, aw0,aw1, m0,m1, log_sigma, pad...]
    pk = nc.dram_tensor("pk", (N, 12), f32, kind="ExternalInput").ap()
    out = nc.dram_tensor("out", (BPC, H, W), f32, kind="ExternalOutput").ap()

    from concourse.tile import add_dep_helper

    with ExitStack() as ctx:
        tc = ctx.enter_context(tile.TileContext(nc))
        consts = ctx.enter_context(tc.tile_pool(name="consts", bufs=1))
        work = ctx.enter_context(tc.tile_pool(name="work", bufs=2))
        small = ctx.enter_context(tc.tile_pool(name="small", bufs=4))
        psum = ctx.enter_context(tc.tile_pool(name="psum", bufs=2, space="PSUM"))

        # ---- constants / per-core inputs ----
        pkt = consts.tile([128, 12], f32)
        nc.sync.dma_start(out=pkt, in_=pk)
        # grid = [0..W) per partition, generated on-chip (prefix scan of ones)
        # so no DMA sits on the critical path; grid2 = grid^2
        ones = consts.tile([128, W], f32)
        nc.vector.memset(ones, 1.0)
        grid = consts.tile([128, W], f32)
        nc.vector.tensor_tensor_scan(grid, ones, ones, -1.0, OP.add, OP.mult)
        grid2 = consts.tile([128, W], f32)
        nc.vector.tensor_mul(grid2, grid, grid)

        # cneg = -0.5 / (exp(2*log_sigma) + 1e-6), replicated on all partitions:
        # reciprocal((sig2 + 1e-6) * -2)
        sig2 = consts.tile([128, 1], f32)
        nc.scalar.activation(sig2, pkt[:, 8:9], AF.Exp, scale=2.0)
        sig2e = consts.tile([128, 1], f32)
        nc.vector.tensor_scalar(sig2e, sig2, 1e-6, -2.0, OP.add, OP.mult)
        cneg = consts.tile([128, 1], f32)
        i_cneg = nc.vector.reciprocal(cneg, sig2e)

        # weights = attn * mask
        wt = consts.tile([128, BPC], f32)
        nc.vector.tensor_mul(wt, pkt[:, 4:6], pkt[:, 6:8])

        # n2 = -2*[u0,u1,v0,v1];  ncrd = -coords;  bias4 = cneg*coord^2
        n2 = consts.tile([128, 4], f32)
        nc.vector.tensor_scalar_mul(n2, pkt[:, 0:4], -2.0)
        ncrd = consts.tile([128, 4], f32)
        nc.vector.tensor_scalar_mul(ncrd, pkt[:, 0:4], -1.0)
        pc2 = consts.tile([128, 4], f32)
        nc.vector.tensor_mul(pc2, pkt[:, 0:4], pkt[:, 0:4])
        bias4 = consts.tile([128, 4], f32)
        i_bias4 = nc.vector.tensor_scalar(bias4, pc2, cneg[:, 0:1], None, OP.mult)

        # PE warm-up: small dependency-free matmuls keep the PE array in its
        # continuous-busy ramp so the real matmuls run at full rate
        pwarm = psum.tile([1, 64], f32, tag="pwarm")
        for _ in range(14):
            nc.tensor.matmul(pwarm, ones[0:1, 0:1], ones[0:1, 0:64])

        # ---- per-sample pipeline ----
        # per-engine staging: ACT writes chunk-0 rows, DVE writes chunk-1 rows;
        # one DMA per staging tile (two total), so no shared-writer stalls
        o_act = consts.tile([HC, BPC, W], f32)
        o_dve = consts.tile([HC, BPC, W], f32)
        prev = {}
        for b in range(BPC):
            # x side first; weight applied on the way to f32r (DVE) while the
            # y-side Exp runs on ACT
            ddx = work.tile([128, W], f32, tag="ddx")
            i_sx = nc.vector.scalar_tensor_tensor(
                ddx, grid, n2[:, b : b + 1], grid2, OP.mult, OP.add
            )
            gx = work.tile([128, W], f32, tag="gx")
            i_expx = nc.scalar.activation(
                gx, ddx, AF.Exp, scale=cneg[:, 0:1], bias=bias4[:, b : b + 1]
            )
            wgx = work.tile([128, WP], f32r, tag="wgx")
            nc.vector.memset(wgx[:, W:WP].bitcast(mybir.dt.uint32), 0)
            i_wgx = nc.vector.tensor_scalar_mul(wgx[:, 0:W], gx, wt[:, b : b + 1])
            # y side exponent on DVE (one STT), Exp on ACT
            ddy = work.tile([128, W], f32, tag="ddy")
            i_sy = nc.vector.scalar_tensor_tensor(
                ddy, grid, n2[:, 2 + b : 3 + b], grid2, OP.mult, OP.add
            )
            add_dep_helper(i_sy.ins, i_cneg.ins, sync=False, reason="cneg first")
            add_dep_helper(i_sy.ins, i_bias4.ins, sync=False, reason="bias4 first")
            gy = work.tile([128, W], f32r, tag="gy")
            i_expy = nc.scalar.activation(
                gy, ddy, AF.Exp, scale=cneg[:, 0:1], bias=bias4[:, 2 + b : 3 + b]
            )
            # keep the tiny cneg/bias4 chain ahead of the fat STTs in the
            # in-order DVE queue
            add_dep_helper(i_sx.ins, i_cneg.ins, sync=False, reason="cneg first")
            add_dep_helper(i_sx.ins, i_bias4.ins, sync=False, reason="bias4 first")

            pmap = psum.tile([HC, 2, WP], f32, tag="pmap")
            nc.tensor.matmul(pmap[:, 0, :], gy[:, 0:HC], wgx)
            nc.tensor.matmul(pmap[:, 1, :], gy[:, HC:H], wgx)

            # per-sample max over the whole map: free-dim reduce on DVE, then
            # partition all-reduce + combine + eps on GPSIMD (one engine hop)
            mrow = small.tile([HC, 1], f32, tag="mrow")
            nc.vector.reduce_max(mrow, pmap[:, :, 0:W], axis=AX.XY)
            mall = small.tile([HC, 1], f32, tag="mall")
            nc.gpsimd.partition_all_reduce(
                mall, mrow, channels=HC, reduce_op=bass_isa.ReduceOp.max
            )
            mxe = small.tile([HC, 1], f32, tag="mxe")
            nc.vector.tensor_scalar_add(mxe, mall, 1e-6)
            rs = small.tile([HC, 1], f32, tag="rs")
            i_recip = nc.vector.reciprocal(rs, mxe)

            nc.scalar.mul(o_act[:, b, :], pmap[:, 0, 0:W], rs[:, 0:1])
            nc.vector.tensor_scalar_mul(o_dve[:, b, :], pmap[:, 1, 0:W], rs[:, 0:1])
            prev = {"wgx": i_wgx, "recip": i_recip, "expx": i_expx, "expy": i_expy}
        ost = out.rearrange("b (c p) w -> p b c w", c=2)
        nc.sync.dma_start(out=ost[:, :, 0, :], in_=o_act)
        nc.sync.dma_start(out=ost[:, :, 1, :], in_=o_dve)

    nc.compile()
    _CACHE["nc"] = nc
    return nc


def kernel(pixel_coords, attn_weights, in_frame_mask, log_sigma, **kwargs):
    pixel_coords = np.asarray(pixel_coords, dtype=np.float32)
    attn_weights = np.asarray(attn_weights, dtype=np.float32)
    mask_f = np.asarray(in_frame_mask).astype(np.float32)
    ls = float(np.asarray(log_sigma, dtype=np.float32))

    nc = _build()
    from concourse.bass_utils import run_bass_kernel_spmd

    in_maps = []
    for i in range(NCORES):
        sl = slice(i * BPC, (i + 1) * BPC)
        pc = pixel_coords[sl]  # (BPC, N, 2)
        aw = attn_weights[sl]  # (BPC, N)
        mf = mask_f[sl]
        pkt = np.zeros((N, 12), dtype=np.float32)
        pkt[:, 0] = pc[0, :, 0]
        pkt[:, 1] = pc[1, :, 0]
        pkt[:, 2] = pc[0, :, 1]
        pkt[:, 3] = pc[1, :, 1]
        pkt[:, 4] = aw[0]
        pkt[:, 5] = aw[1]
        pkt[:, 6] = mf[0]
        pkt[:, 7] = mf[1]
        pkt[:, 8] = ls
        in_maps.append({"pk": pkt})
    res = run_bass_kernel_spmd(nc, in_maps, core_ids=list(range(NCORES)))
    return np.concatenate([r["out"] for r in res.results], axis=0)


# revision 66
# speedup vs baseline: 1.3089x; 1.0308x over previous
"""Trainium2 Bass kernel for AudioAttentionMapGenerator.

Math (reference):
    sigma = exp(log_sigma); c = 0.5 / (sigma^2 + 1e-6)
    w_n   = attn_weights * mask
    map[b,h,w] = sum_n w_n * exp(-c*((h-v_bn)^2 + (w-u_bn)^2))
    out = map / (max_hw(map) + 1e-6)

The Gaussian is separable: per sample  map = Gy^T @ (w * Gx)  — two
(112,N)@(N,W) matmuls.  Per gaussian axis the exponent is expanded as
    -c (x-u)^2 = cneg*(x^2 - 2 u x) + cneg*u^2
so each gaussian row block is ONE DVE scalar_tensor_tensor
(d = grid*(-2u) + grid2; grid is generated on-chip by a prefix scan, so no
DMA sits on the critical path) followed by ONE activation
Exp(scale=cneg, bias=cneg*u^2).

Sharding: data-parallel over B=16 across 8 cores (2 samples/core), N=128 on
SBUF partitions, H split 112+112.  Matmuls run as float32r with the moving
free dim padded to 256 (full rate per the PE cost model); a burst of tiny
dependency-free matmuls beforehand keeps the PE array in its continuous-busy
ramp so the real matmuls hit the warm clock.  Per-sample max: one fused
free-dim reduce (DVE) + partition all-reduce (GPSIMD); normalization scales
split ACT/DVE into two per-engine staging tiles -> two output DMAs.
"""

import sys

import numpy as np

if "/opt/trn_rl_repo" not in sys.path:
    sys.path.insert(0, "/opt/trn_rl_repo")

B, N, H, W = 16, 128, 224, 224
NCORES = 8
BPC = B // NCORES  # samples per core
HC = H // 2  # 112 — H chunk (stationary free-dim <= 128)
WP = 256  # moving operand padded width (float32r full-rate needs >=256)

_CACHE = {}


def _build():
    if "nc" in _CACHE:
        return _CACHE["nc"]

    from contextlib import ExitStack

    import concourse.bass_isa as bass_isa
    import concourse.tile as tile
    from concourse import bacc, mybir

    f32 = mybir.dt.float32
    f32r = mybir.dt.float32r
    AF = mybir.ActivationFunctionType
    AX = mybir.AxisListType
    OP = mybir.AluOpType

    nc = bacc.Bacc(
        "TRN2",
        target_bir_lowering=False,
        debug=False,
        enable_asserts=False,
        num_devices=NCORES,
    )
    # packed per-core input: [u0,u1,v0,v1, aw0,aw1, m0,m1, log_sigma, pad...]
    pk = nc.dram_tensor("pk", (N, 12), f32, kind="ExternalInput").ap()
    out = nc.dram_tensor("out", (BPC, H, W), f32, kind="ExternalOutput").ap()

    from concourse.tile import add_dep_helper

    with ExitStack() as ctx:
        tc = ctx.enter_context(tile.TileContext(nc))
        consts = ctx.enter_context(tc.tile_pool(name="consts", bufs=1))
        work = ctx.enter_context(tc.tile_pool(name="work", bufs=2))
        small = ctx.enter_context(tc.tile_pool(name="small", bufs=4))
        psum = ctx.enter_context(tc.tile_pool(name="psum", bufs=2, space="PSUM"))

        # ---- constants / per-core inputs ----
        pkt = consts.tile([128, 12], f32)
        nc.sync.dma_start(out=pkt, in_=pk)
        # grid = [0..W) per partition, generated on-chip (prefix scan of ones)
        # so no DMA sits on the critical path; grid2 = grid^2
        ones = consts.tile([128, W], f32)
        nc.vector.memset(ones, 1.0)
        grid = consts.tile([128, W], f32)
        nc.vector.tensor_tensor_scan(grid, ones, ones, -1.0, OP.add, OP.mult)
        grid2 = consts.tile([128, W], f32)
        nc.vector.tensor_mul(grid2, grid, grid)

        # cneg = -0.5 / (exp(2*log_sigma) + 1e-6), replicated on all partitions:
        # reciprocal((sig2 + 1e-6) * -2)
        sig2 = consts.tile([128, 1], f32)
        nc.scalar.activation(sig2, pkt[:, 8:9], AF.Exp, scale=2.0)
        sig2e = consts.tile([128, 1], f32)
        nc.vector.tensor_scalar(sig2e, sig2, 1e-6, -2.0, OP.add, OP.mult)
        cneg = consts.tile([128, 1], f32)
        i_cneg = nc.vector.reciprocal(cneg, sig2e)

        # weights = attn * mask
        wt = consts.tile([128, BPC], f32)
        nc.vector.tensor_mul(wt, pkt[:, 4:6], pkt[:, 6:8])

        # n2 = -2*[u0,u1,v0,v1];  ncrd = -coords;  bias4 = cneg*coord^2
        n2 = consts.tile([128, 4], f32)
        nc.vector.tensor_scalar_mul(n2, pkt[:, 0:4], -2.0)
        ncrd = consts.tile([128, 4], f32)
        nc.vector.tensor_scalar_mul(ncrd, pkt[:, 0:4], -1.0)
        pc2 = consts.tile([128, 4], f32)
        nc.vector.tensor_mul(pc2, pkt[:, 0:4], pkt[:, 0:4])
        bias4 = consts.tile([128, 4], f32)
        i_bias4 = nc.vector.tensor_scalar(bias4, pc2, cneg[:, 0:1], None, OP.mult)

        # PE warm-up: small dependency-free matmuls keep the PE array in its
        # continuous-busy ramp so the real matmuls run at full rate
        pwarm = psum.tile([1, 64], f32, tag="pwarm")
        for _ in range(14):
            nc.tensor.matmul(pwarm, ones[0:1, 0:1], ones[0:1, 0:64])

        # ---- per-sample pipeline ----
        prev = {}
        for b in range(BPC):
            # x side first; weight applied on the way to f32r (DVE) while the
            # y-side Exp runs on ACT
            ddx = work.tile([128, W], f32, tag="ddx")
            i_sx = nc.vector.scalar_tensor_tensor(
                ddx, grid, n2[:, b : b + 1], grid2, OP.mult, OP.add
            )
            gx = work.tile([128, W], f32, tag="gx")
            i_expx = nc.scalar.activation(
                gx, ddx, AF.Exp, scale=cneg[:, 0:1], bias=bias4[:, b : b + 1]
            )
            wgx = work.tile([128, WP], f32r, tag="wgx")
            nc.vector.memset(wgx[:, W:WP].bitcast(mybir.dt.uint32), 0)
            i_wgx = nc.vector.tensor_scalar_mul(wgx[:, 0:W], gx, wt[:, b : b + 1])
            # y side exponent on DVE (one STT), Exp on ACT
            ddy = work.tile([128, W], f32, tag="ddy")
            i_sy = nc.vector.scalar_tensor_tensor(
                ddy, grid, n2[:, 2 + b : 3 + b], grid2, OP.mult, OP.add
            )
            add_dep_helper(i_sy.ins, i_cneg.ins, sync=False, reason="cneg first")
            add_dep_helper(i_sy.ins, i_bias4.ins, sync=False, reason="bias4 first")
            gy = work.tile([128, W], f32r, tag="gy")
            i_expy = nc.scalar.activation(
                gy, ddy, AF.Exp, scale=cneg[:, 0:1], bias=bias4[:, 2 + b : 3 + b]
            )
            # keep the tiny cneg/bias4 chain ahead of the fat STTs in the
            # in-order DVE queue
            add_dep_helper(i_sx.ins, i_cneg.ins, sync=False, reason="cneg first")
            add_dep_helper(i_sx.ins, i_bias4.ins, sync=False, reason="bias4 first")

            pmap = psum.tile([HC, 2, WP], f32, tag="pmap")
            nc.tensor.matmul(pmap[:, 0, :], gy[:, 0:HC], wgx)
            nc.tensor.matmul(pmap[:, 1, :], gy[:, HC:H], wgx)

            # per-sample max over the whole map: free-dim reduce on DVE, then
            # partition all-reduce + combine + eps on GPSIMD (one engine hop)
            mrow = small.tile([HC, 1], f32, tag="mrow")
            nc.vector.reduce_max(mrow, pmap[:, :, 0:W], axis=AX.XY)
            mall = small.tile([HC, 1], f32, tag="mall")
            nc.gpsimd.partition_all_reduce(
                mall, mrow, channels=HC, reduce_op=bass_isa.ReduceOp.max
            )
            mxe = small.tile([HC, 1], f32, tag="mxe")
            nc.vector.tensor_scalar_add(mxe, mall, 1e-6)
            rs = small.tile([HC, 1], f32, tag="rs")
            i_recip = nc.vector.reciprocal(rs, mxe)

            o0 = work.tile([HC, W], f32, tag="o0")
            nc.scalar.mul(o0, pmap[:, 0, 0:W], rs[:, 0:1])
            o1 = work.tile([HC, W], f32, tag="o1")
            nc.scalar.mul(o1, pmap[:, 1, 0:W], rs[:, 0:1])
            nc.sync.dma_start(out=out[b, 0:HC, :], in_=o0)
            nc.sync.dma_start(out=out[b, HC:H, :], in_=o1)
            prev = {"wgx": i_wgx, "recip": i_recip, "expx": i_expx, "expy": i_expy}

    nc.compile()
    _CACHE["nc"] = nc
    return nc


def kernel(pixel_coords, attn_weights, in_frame_mask, log_sigma, **kwargs):
    pixel_coords = np.asarray(pixel_coords, dtype=np.float32)
    attn_weights = np.asarray(attn_weights, dtype=np.float32)
    mask_f = np.asarray(in_frame_mask).astype(np.float32)
    ls = float(np.asarray(log_sigma, dtype=np.float32))

    nc = _build()
    from concourse.bass_utils import run_bass_kernel_spmd

    in_maps = []
    for i in range(NCORES):
        sl = slice(i * BPC, (i + 1) * BPC)
        pc = pixel_coords[sl]  # (BPC, N, 2)
        aw = attn_weights[sl]  # (BPC, N)
        mf = mask_f[sl]
        pkt = np.zeros((N, 12), dtype=np.float32)
        pkt[:, 0] = pc[0, :, 0]
        pkt[:, 1] = pc[1, :, 0]
        pkt[:, 2] = pc[0, :, 1]
        pkt[:, 3] = pc[1, :, 1]
        pkt[:, 4] = aw[0]
        pkt[:, 5] = aw[1]
        pkt[:, 6] = mf[0]
        pkt[:, 7] = mf[1]
        pkt[:, 8] = ls
        in_maps.append({"pk": pkt})
    res = run_bass_kernel_spmd(nc, in_maps, core_ids=list(range(NCORES)))
    return np.concatenate([r["out"] for r in res.results], axis=0)
